# revision 23
# baseline (speedup 1.0000x reference)
"""CrossTuckerLayer kernel for 8x Trainium2 NeuronCores (Bass/Tile).

Computes y = einsum('bnvade,ABCDEF,oA,pB,qC,aD,dE,eF->bnvopq', ...)
reshaped to [b, n, v, o*p, q], data-parallel over the 2048 (b,n,v) samples
(256 per core). All HBM I/O is bf16 (harness gate is rel_err < 2e-2; this
path lands ~3.4e-3), halving DMA traffic vs fp32.

Host folds the tiny Tucker factors (all <10K params) into two matrices:
  M    [16384, 8] = einsum('ABCDEF,aD,dE,eF->adeABC', core, a0, a1, a2)
  Wout [8, 32768] = einsum('oA,pB,qC->ABCopq', u0, u1, u2)

Per core the 256 samples split into two 128-sample windows:
  stage A (PE): s2_w[8, 128] = sum over 128 fin-chunks of
      M_ck[128f, 8]^T @ x_ck[128f, 128s].
  s2 is downcast then replicated to partition blocks 0/32/64/96 (three
      tiny gpsimd-SWDGE SBUF->SBUF DMAs: their own queue + semaphores,
      so the scheduler never serializes C(w0) behind the HWDGE pool).
  stage C (PE): y[128s, 512] tiles = s2_w[8, 128]^T @ W[8, 512], K=8.
      Stage st alternates PE row-groups (2st)%4 / (2st+1)%4 chunk by
      chunk via tile_position=(32g, 0) — adjacent matmuls in different
      groups overlap PE fill/drain, and stage 0 only needs the downcast
      plus the first replicate. Chunk pairs write the two banks of a
      [128, 1024] PSUM tile so each PSUM->SBUF copy moves 1024 cols
      (~1.2-1.4ns/col; only vector and scalar can read PSUM on TRN2,
      alternating pair-tiles). Wout ships as a 0.5MB group-interleaved
      [8, 4*8192] tensor DMA'd straight into the four 32-row SBUF slots.

Schedule. The problem is HBM-bound (~8MB x read + 16MB y write + 0.75MB
weights per core). Measured DMA behavior: per-descriptor service scales
with the per-partition contiguous run size (~6KB run -> ~40GB/s, 16KB
-> ~80, aggregate ~400GB/s), service is round-robin so concurrent
same-size descriptors complete together, and software-DGE traffic
stalls early hardware-queue service. Hence:
  - x(w0): a tiny 8-chunk starter (first A matmul ~11us in; M is split
    so its first chunks land even earlier) plus five 24-chunk tiles.
  - x(w1): two upfront tiles (holding the LATE w1 chunks) keep queue
    depth through the w0 tail; the rest ring-reuse w0 buffers so their
    issue releases as stage A consumes the matching w0 tile. All
    ring-deferred issues live on sync so scalar's stream stays pure
    upfront-issues + copies (no head-of-line blocking of C copies).
  - y: per 8192-col stage, two 4096-col half-DMAs (8KB contiguous per
    row), h0 on sync, h1 on scalar; yp bufs=5 so y-DMA completion
    latency never throttles the copies.
  - A(w1) slices are emitted between C(w0) stages 1..3; the PE weaves
    them into the psC-ring idle gaps (psA bufs=2).
"""

import numpy as np
import ml_dtypes

import concourse.bass as bass
import concourse.bacc as bacc
import concourse.mybir as mybir
from concourse.tile import TileContext
from concourse.bass_utils import run_bass_kernel_spmd

F32 = mybir.dt.float32
BF16 = mybir.dt.bfloat16
BF = ml_dtypes.bfloat16

NCORES = 8
S_TOT = 2048          # 4*64*8 samples
S = S_TOT // NCORES   # 256 per core
FIN = 16 * 16 * 64    # 16384
FOUT = 256 * 128      # 32768
NCK = FIN // 128      # 128 contraction chunks of 128
WIN = 128             # samples per window
N_WIN = S // WIN      # 2
YCHUNK = 512          # one matmul's psum cols (fits a 2KB fp32 bank)
YSTAGE = 8192         # cols per y staging tile (two 4096-col DMA halves)
N_YSTAGE = FOUT // YSTAGE  # 4 per window
NTILE = 4             # concurrent row-group matmuls in stage C
NSLOT = FOUT // YCHUNK // NTILE  # 16 column slots per row-group

# x tiles: (engine, window, ck0, nck, tag). Tags ra/rb are 3-deep rings
# (sync / scalar); w1 tiles reuse w0 buffers so their issue releases as
# stage A consumes the matching w0 tile.
X_TILES = [
    ("sync", 0, 0, 8, "x0"),        # starter
    ("scalar", 0, 8, 24, "pa"),
    ("sync", 0, 32, 24, "pb"),
    ("scalar", 0, 56, 24, "pa"),
    ("sync", 0, 80, 24, "pb"),
    ("scalar", 0, 104, 24, "pa"),
    # upfront w1 tiles hold LATE w1 chunks (consumed by late A(w1)
    # matmuls) and keep queue depth through the w0 tail
    ("scalar", 1, 72, 24, "pc"),
    ("sync", 1, 96, 24, "pc"),
    # ring-deferred w1 issues all live on SYNC: the scalar engine's
    # stream stays pure upfront-issues + copies, so C(w0) copies are
    # never head-of-line blocked behind a deferred issue
    ("sync", 1, 0, 24, "pa"),       # waits x(0,8) consumed
    ("sync", 1, 24, 24, "pb"),      # waits x(0,32) consumed
    ("sync", 1, 48, 24, "pa"),      # waits x(0,56) consumed
    ("sync", 1, 120, 8, "x0"),      # waits x(0,0) consumed
]
MM_SPLITS = [(0, 8), (8, 120)]


def _host_weights(core, u0, u1, u2, a0, a1, a2):
    """Fold the Tucker factors into M [128f, 128ck*8] and the
    group-permuted Wout wl_g [8, 4*NSLOT*512]."""
    M = np.einsum(
        "ABCDEF,aD,dE,eF->adeABC",
        core.astype(np.float64), a0.astype(np.float64),
        a1.astype(np.float64), a2.astype(np.float64),
    ).reshape(FIN, 8)
    # SBUF layout [f, ck*8 + r] where fin = ck*128 + f
    Mdev = np.ascontiguousarray(
        M.reshape(NCK, 128, 8).transpose(1, 0, 2).reshape(128, NCK * 8)
    ).astype(BF)

    Wout = np.einsum(
        "oA,pB,qC->ABCopq",
        u0.astype(np.float64), u1.astype(np.float64), u2.astype(np.float64),
    ).reshape(8, FOUT)
    # stage st alternates PE row-groups (2st)%4 / (2st+1)%4 chunk by
    # chunk (adjacent matmuls in different groups overlap fill/drain;
    # stage 0 needs only the s2 downcast + the first replicate DMA).
    # Group g's SBUF slot packs its 16 chunks: slot = (st>=2)*8 + j//2.
    wl_g = np.zeros((8, NTILE * NSLOT * YCHUNK), dtype=np.float64)
    for st in range(4):
        for j in range(16):
            g = (2 * st + (j % 2)) % 4
            slot = (8 if st >= 2 else 0) + j // 2
            wl_g[:, g * NSLOT * YCHUNK + slot * YCHUNK:
                 g * NSLOT * YCHUNK + (slot + 1) * YCHUNK] = \
                Wout[:, (st * 16 + j) * YCHUNK:(st * 16 + j + 1) * YCHUNK]
    return Mdev, np.ascontiguousarray(wl_g.astype(BF))


def _host_x(x):
    """x [2048, FIN] f32 -> per-core dev layout [128f, w*16K + ck*128 + s]."""
    xb = x.reshape(S_TOT, FIN).astype(BF)
    xd = np.ascontiguousarray(
        xb.reshape(NCORES, N_WIN, WIN, NCK, 128).transpose(0, 4, 1, 3, 2)
    ).reshape(NCORES, 128, N_WIN * FIN)
    return xd


def _build():
    nc = bacc.Bacc("TRN2", target_bir_lowering=False, debug=False)
    x_d = nc.dram_tensor("x", [128, N_WIN * FIN], BF16, kind="ExternalInput")
    m_d = nc.dram_tensor("m", [128, NCK * 8], BF16, kind="ExternalInput")
    wl_d = nc.dram_tensor("wl", [8, NTILE * NSLOT * YCHUNK], BF16,
                          kind="ExternalInput")
    y_d = nc.dram_tensor("y", [S, FOUT], BF16, kind="ExternalOutput")

    with TileContext(nc) as tc:
        with (
            tc.tile_pool(name="consts", bufs=1) as cpool,
            tc.tile_pool(name="xs", bufs=1) as xs,    # tiny starter ring
            tc.tile_pool(name="xa", bufs=3) as xa,    # 24-chunk ring A
            tc.tile_pool(name="xb", bufs=2) as xb,    # 24-chunk ring B
            tc.tile_pool(name="xc", bufs=2) as xc,    # upfront w1 tiles
            tc.tile_pool(name="s2p", bufs=2) as s2p,
            tc.tile_pool(name="yp", bufs=5) as yp,
            tc.tile_pool(name="psA", bufs=2, space=bass.MemorySpace.PSUM) as psA,
            tc.tile_pool(name="psC", bufs=3, space=bass.MemorySpace.PSUM) as psC,
        ):
            # M slivers so A's first chunks aren't gated on the whole M.
            mm_tiles = []
            for (ck0, n) in MM_SPLITS:
                mmt = cpool.tile([128, n * 8], BF16, name=f"mm_{ck0}")
                nc.scalar.dma_start(mmt[:], m_d[:, ck0 * 8:(ck0 + n) * 8])
                mm_tiles.append((ck0, n, mmt))

            def mm_for(ck):
                for (ck0, n, t) in mm_tiles:
                    if ck0 <= ck < ck0 + n:
                        return t[:, (ck - ck0) * 8:(ck - ck0 + 1) * 8]
                raise AssertionError(ck)

            pools = {"x0": xs, "pa": xa, "pb": xb, "pc": xc}
            x_tiles = {}
            for (eng, w, ck0, n, tag) in X_TILES:
                xg = pools[tag].tile([128, n * WIN], BF16, tag=tag,
                                     name=f"x_{w}_{ck0}")
                getattr(nc, eng).dma_start(
                    xg[:],
                    x_d[:, (w * NCK + ck0) * WIN:(w * NCK + ck0 + n) * WIN],
                )
                x_tiles[(w, ck0)] = xg

            def x_for(w, ck):
                for (eng, ww, ck0, n, tag) in X_TILES:
                    if ww == w and ck0 <= ck < ck0 + n:
                        xg = x_tiles[(w, ck0)]
                        return xg[:, (ck - ck0) * WIN:(ck - ck0 + 1) * WIN]
                raise AssertionError((w, ck))

            # Wout straight into the four 32-row SBUF slots (HWDGE;
            # software DGE stalls the early hardware-queue service).
            wl = cpool.tile([128, NSLOT * YCHUNK], BF16)
            for i in range(NTILE):
                eng = nc.sync if i % 2 == 0 else nc.scalar
                eng.dma_start(
                    wl[32 * i:32 * i + 8, :],
                    wl_d[:, i * NSLOT * YCHUNK:(i + 1) * NSLOT * YCHUNK],
                )

            sA = [psA.tile([8, WIN], F32, tag="sA", name=f"sA_{w}")
                  for w in range(N_WIN)]
            s2r = [s2p.tile([128, WIN], BF16, tag="s2", name=f"s2_{w}")
                   for w in range(N_WIN)]

            def emit_a_slice(w, ck0, n):
                for ck in range(ck0, ck0 + n):
                    nc.tensor.matmul(
                        sA[w][:],
                        mm_for(ck),
                        x_for(w, ck),
                        start=(ck == 0), stop=(ck == NCK - 1),
                        skip_group_check=True,
                    )

            def emit_s2_replicate(w):
                # bf16 downcast into row-group 0, then fan out to 32/64/96
                # via gpsimd SWDGE: tiny transfers with their own queue +
                # semaphores, so they never wait on the HWDGE sem pool
                # (which the scheduler sim models as backed up behind the
                # x stream, pushing C(w0) behind all of A(w1)).
                nc.vector.tensor_copy(s2r[w][0:8, :], sA[w][:])
                nc.gpsimd.dma_start(s2r[w][32:40, :], s2r[w][0:8, :])
                nc.gpsimd.dma_start(s2r[w][64:72, :], s2r[w][0:8, :])
                nc.gpsimd.dma_start(s2r[w][96:104, :], s2r[w][0:8, :])

            def emit_c_stage(w, st):
                ctx_hp = tc.high_priority()
                ctx_hp.__enter__()
                # stage st alternates row-groups (2st)%4 / (2st+1)%4 so
                # adjacent matmuls overlap PE fill/drain; stage 0 needs
                # only s2r[0:8] (downcast) + the first replicate DMA.
                sl0 = 8 if (st % N_YSTAGE) >= 2 else 0
                y_sb = yp.tile([128, YSTAGE], BF16, tag="ysb", name="y_sb")
                for jj in range(8):
                    # chunk pair (2jj, 2jj+1) -> one 2-bank PSUM tile
                    y_ps = psC.tile([128, 2 * YCHUNK], F32, tag="yps",
                                    name="y_ps")
                    for q in range(2):
                        g = (2 * (st % N_YSTAGE) + q) % NTILE
                        slot = sl0 + jj
                        nc.tensor.matmul(
                            y_ps[:, q * YCHUNK:(q + 1) * YCHUNK],
                            s2r[w][32 * g:32 * g + 8, :],
                            wl[32 * g:32 * g + 8,
                               slot * YCHUNK:(slot + 1) * YCHUNK],
                            start=True, stop=True,
                            tile_position=(32 * g, 0),
                        )
                    dst = y_sb[:, 2 * jj * YCHUNK:(2 * jj + 2) * YCHUNK]
                    # alternate engines per pair-tile so the psC ring
                    # turns over at the two engines' combined pace
                    if jj % 2 == 0:
                        nc.vector.tensor_copy(dst, y_ps[:])
                    else:
                        nc.scalar.copy(dst, y_ps[:])
                half = 4096
                nc.sync.dma_start(
                    y_d[w * WIN:(w + 1) * WIN,
                        st * YSTAGE:st * YSTAGE + half],
                    y_sb[:, 0:half],
                )
                nc.scalar.dma_start(
                    y_d[w * WIN:(w + 1) * WIN,
                        st * YSTAGE + half:(st + 1) * YSTAGE],
                    y_sb[:, half:YSTAGE],
                )
                ctx_hp.__exit__(None, None, None)

            # stage A w0 chases the x stream; A(w1) slices interleave
            # with C(w0) stages 1..3.
            for (eng, w, ck0, n, tag) in X_TILES[:6]:
                emit_a_slice(0, ck0, n)
            emit_s2_replicate(0)
            emit_c_stage(0, 0)
            emit_c_stage(0, 1)
            emit_a_slice(1, 0, 24)
            emit_a_slice(1, 24, 24)
            emit_c_stage(0, 2)
            emit_a_slice(1, 48, 24)
            emit_a_slice(1, 72, 24)
            emit_c_stage(0, 3)
            emit_a_slice(1, 96, 24)
            emit_a_slice(1, 120, 8)
            emit_s2_replicate(1)
            for st in range(N_YSTAGE):
                emit_c_stage(1, st)
    nc.compile()
    return nc


_NC_CACHE = []


def _get_nc():
    if not _NC_CACHE:
        _NC_CACHE.append(_build())
    return _NC_CACHE[0]


def run(inputs, trace=False):
    x = np.asarray(inputs["x"], dtype=np.float32)
    Mdev, wl_g = _host_weights(
        np.asarray(inputs["core"]),
        np.asarray(inputs["u0"]), np.asarray(inputs["u1"]),
        np.asarray(inputs["u2"]),
        np.asarray(inputs["a0"]), np.asarray(inputs["a1"]),
        np.asarray(inputs["a2"]),
    )
    xd = _host_x(x)
    nc = _get_nc()
    in_maps = []
    for i in range(NCORES):
        in_maps.append({
            "x": xd[i],
            "m": Mdev,
            "wl": wl_g,
        })
    res = run_bass_kernel_spmd(
        nc, in_maps, core_ids=list(range(NCORES)), trace=trace,
    )
    y = np.concatenate([np.asarray(r["y"]) for r in res.results], axis=0)
    y = y.astype(np.float32).reshape(4, 64, 8, 256, 128)
    return y, res


def kernel(**inputs) -> np.ndarray:
    y, _ = run(inputs, trace=False)
    return y
